# revision 43
# baseline (speedup 1.0000x reference)
"""Trainium2 Bass kernel for masked multi-head attention.

Problem: B=4, S=2048, D=768, H=12 (head_dim=64), boolean prune mask per
head, softmax over keys, out-projection.

Sharding (8 cores): data-parallel over batch (4) x tensor-parallel over
head halves (2 x 6 heads).  Core c handles batch c//2 and heads
(c%2)*6 .. (c%2)*6+5.  Each core computes its 6 heads' QKV projections,
attention, and the partial out-projection (row-parallel slice of out_w).
The host sums the two partials per batch and adds out_b (the standard
tensor-parallel reduce, done during unshard).

On-chip layout choices:
  * Activations are kept feature-major ("transposed"): hsT [769, 2048]
    (row 768 = ones for the affine/bias trick), qT/kT [384, 2048].
  * Scores are computed directly transposed: S_T[k, q] = kT.T-slice @ qT,
    so P.T is exactly the rhs the ctx matmul needs -> no transposes.
  * The mask is pre-transposed to [k, q] per head on the host (bf16 0/1),
    applied multiplicatively after exp (equivalent to -inf before
    softmax, and safe: |scores/8| < ~2 so no overflow without max-sub).
  * V carries an appended ones column per head (wvT has 6x65 columns)
    so row 64 of each ctx PSUM accumulates the softmax denominators.
  * Normalization: denominators gathered per head via a casting SWDGE
    DMA, one batched DVE reciprocal at the end, broadcast across each
    head's 64 partitions with a selector matmul, then DVE multiplies.
  * All matmul inputs bf16 (PSUM accumulates f32); output stored bf16.
  * Scores/ctx stationaries zero-padded to K=128 / M=128 (half-active
    PE arrays made the HAM clock gate hold the PE at 1.2 GHz), plus a
    PE warm-up spin during the initial DMAs.
"""

import os
import sys
import math

import numpy as np

try:
    import concourse.bass as bass
except ImportError:  # pragma: no cover - path fallback for fresh dirs
    for _p in ("/opt/trn_rl_repo", "/root/.axon_site/_ro/trn_rl_repo"):
        if os.path.isdir(_p) and _p not in sys.path:
            sys.path.insert(0, _p)
    import concourse.bass as bass

import ml_dtypes
import concourse.mybir as mybir
from concourse import bacc
from concourse.tile import TileContext
from concourse.bass_utils import run_bass_kernel_spmd

BF16 = ml_dtypes.bfloat16
F32 = mybir.dt.float32
BBF = mybir.dt.bfloat16

B, S, D, H = 4, 2048, 768, 12
HD = 64          # head dim
HPC = 6          # heads per core
FPC = HPC * HD   # features per core (384)
NCORES = 8
KT = S // 128    # 16 key tiles
ST = S // 128    # 16 seq tiles

_CACHE = {}
_last_result = None


def _build_bass():
    nc = bacc.Bacc()

    hsT = nc.declare_dram_parameter("hsT", [D + 1, S], BBF, isOutput=False)
    wqT = nc.declare_dram_parameter("wqT", [D + 1, FPC], BBF, isOutput=False)
    wkT = nc.declare_dram_parameter("wkT", [D + 1, FPC], BBF, isOutput=False)
    wvT = nc.declare_dram_parameter("wvT", [D + 1, HPC * (HD + 1)], BBF, isOutput=False)
    owT = nc.declare_dram_parameter("owT", [FPC, D], BBF, isOutput=False)
    selp = nc.declare_dram_parameter("selp", [128, FPC], BBF, isOutput=False)
    mT = nc.declare_dram_parameter("mT", [HPC, KT, 128, S], BBF, isOutput=False)
    out = nc.declare_dram_parameter("out", [S, D], BBF, isOutput=True)

    EXP = mybir.ActivationFunctionType.Exp
    LN = mybir.ActivationFunctionType.Ln
    MULT = mybir.AluOpType.mult

    with TileContext(nc) as tc, \
            tc.tile_pool(name="persist", bufs=1) as pp, \
            tc.tile_pool(name="maskp", bufs=3) as mask_pool, \
            tc.tile_pool(name="pbuf", bufs=3) as p_pool, \
            tc.tile_pool(name="obuf", bufs=2) as o_pool, \
            tc.tile_pool(name="pswork", bufs=2, space="PSUM") as ps_pool, \
            tc.tile_pool(name="psctx", bufs=1, space="PSUM") as ctx_pool:

        # ---------------- persistent SBUF tensors + input DMAs ----------
        hsT_sb = [pp.tile([128, S], BBF, name=f"hsT{c}", tag=f"hsT{c}")
                  for c in range(6)]
        # all-ones row: content is uniform, so a [1, 512] tile serves every
        # 512-wide rhs slice and every 128-wide lhsT slice
        ones_sb = pp.tile([1, 512], BBF, name="ones_row", tag="ones_row")
        for c in range(6):
            eng = nc.sync if c % 2 == 0 else nc.scalar
            eng.dma_start(out=hsT_sb[c], in_=hsT[c * 128:(c + 1) * 128, :])
        nc.sync.dma_start(out=ones_sb, in_=hsT[D:D + 1, 0:512])

        def load_w(handle, width, nm):
            tiles = [pp.tile([128, width], BBF, name=f"{nm}{c}", tag=f"{nm}{c}")
                     for c in range(6)]
            brow = pp.tile([1, width], BBF, name=f"{nm}b", tag=f"{nm}b")
            for c in range(6):
                eng = nc.scalar if c % 2 == 0 else nc.sync
                eng.dma_start(out=tiles[c],
                              in_=handle[c * 128:(c + 1) * 128, :])
            nc.scalar.dma_start(out=brow, in_=handle[D:D + 1, :])
            return tiles, brow

        wq_sb, wqb_sb = load_w(wqT, FPC, "wq")
        wk_sb, wkb_sb = load_w(wkT, FPC, "wk")
        wv_sb, wvb_sb = load_w(wvT, HPC * (HD + 1), "wv")

        ow_sb = [pp.tile([128, D], BBF, name=f"ow{c}", tag=f"ow{c}")
                 for c in range(3)]
        for c in range(3):
            nc.scalar.dma_start(out=ow_sb[c], in_=owT[c * 128:(c + 1) * 128, :])
        sel_sb = pp.tile([128, FPC], BBF, name="sel", tag="sel")
        nc.scalar.dma_start(out=sel_sb, in_=selp[:, :])

        # Per-head zero-padded qT/kT [128, S]: rows 0-63 = head features,
        # rows 64-127 = 0.  K=128 scores matmuls keep the PE array fully
        # active (K=64 ran at HAM half-clock) and zeros contribute nothing.
        qTz = [pp.tile([128, S], BBF, name=f"qTz{h}", tag=f"qTz{h}")
               for h in range(HPC)]
        kTz = [pp.tile([128, S], BBF, name=f"kTz{h}", tag=f"kTz{h}")
               for h in range(HPC)]
        # v6pad [128, 6*128]: head h occupies cols h*128..h*128+64 (64 v cols
        # + ones col), cols 65-127 of each block zero -> ctx lhsT is a full
        # [128, 128] stationary.
        v6_sb = [pp.tile([128, HPC * 128], BBF, name=f"v6_{t}", tag=f"v6_{t}")
                 for t in range(ST)]
        # PE warm-up: dummy matmuls on (uninitialized, never-read) data so
        # the HAM clock gate reaches 8/8 while the initial DMAs land.  No
        # deps -> starts right after the preamble.
        warm_sb = pp.tile([128, 512], BBF, name="warm_sb", tag="warm_sb")
        nc.vector.memset(warm_sb, 0.0)
        warm_ps = ps_pool.tile([128, 512], F32, tag="work", name="warm_ps")
        for _ in range(56):
            nc.tensor.matmul(warm_ps, lhsT=warm_sb[:, 0:128], rhs=warm_sb,
                             start=True, stop=True)
        nc.vector.tensor_copy(out=warm_sb[:, 0:1], in_=warm_ps[:, 0:1])

        # Zero only what must be zero, off the DVE critical path:
        # qTz/kTz rows 64-127 on GpSimd (head order, so head 0 unblocks
        # first); v6's 63-wide per-head pad columns via a tiny strided DVE
        # memset.
        for h in range(HPC):
            nc.gpsimd.memset(qTz[h][64:128, :], 0.0)
            nc.gpsimd.memset(kTz[h][64:128, :], 0.0)
        for t in range(ST):
            nc.vector.memset(
                v6_sb[t].rearrange("p (h c) -> p h c", c=128)[:, :, HD + 1:], 0.0)
        ctxu_sb = [pp.tile([65, S], BBF, name=f"ctxu{h}", tag=f"ctxu{h}")
                   for h in range(HPC)]
        ctxa_sb = [pp.tile([128, S], BBF, name=f"ctxa{t}", tag=f"ctxa{t}")
                   for t in range(3)]

        # ---------------- projection emitters ----------------------------
        _prew = [0]

        def _prew_tile(nm):
            # rotate projection psums over 3 slots: the 2 "work" slots plus
            # the (idle until attention) ctx slot -> denser prework PE
            _prew[0] += 1
            if _prew[0] % 3 == 0:
                return ctx_pool.tile([128, 512], F32, tag="ctx", name=nm)
            return ps_pool.tile([128, 512], F32, tag="work", name=nm)

        def qk_chunk(w_tiles, w_brow, dst, t, nb):
            # one [128, 512] psum tile of the q or k projection for feature
            # M-tile t (heads 2t, 2t+1), seq block nb; split-copy the two
            # heads' 64-row halves into their zero-padded dsts.
            ps = _prew_tile(f"qkps{id(dst)}_{t}_{nb}")
            for c in range(6):
                nc.tensor.matmul(
                    ps,
                    lhsT=w_tiles[c][:, t * 128:(t + 1) * 128],
                    rhs=hsT_sb[c][:, nb * 512:(nb + 1) * 512],
                    start=(c == 0), stop=False)
            nc.tensor.matmul(
                ps,
                lhsT=w_brow[:, t * 128:(t + 1) * 128],
                rhs=ones_sb,
                start=False, stop=True)
            ns = slice(nb * 512, (nb + 1) * 512)
            nc.vector.tensor_copy(out=dst[2 * t][0:64, ns], in_=ps[0:64, :])
            nc.scalar.copy(out=dst[2 * t + 1][0:64, ns], in_=ps[64:128, :])

        def v_tile(t):
            VW = HPC * (HD + 1)  # 390
            ps = _prew_tile(f"vps{t}")[:, 0:VW]
            for c in range(6):
                nc.tensor.matmul(
                    ps,
                    lhsT=hsT_sb[c][:, t * 128:(t + 1) * 128],
                    rhs=wv_sb[c],
                    start=(c == 0), stop=False)
            nc.tensor.matmul(
                ps,
                lhsT=ones_sb[:, 0:128],
                rhs=wvb_sb,
                start=False, stop=True)
            # scatter [128, 6, 65] -> cols 0..64 of each 128-wide head block
            nc.vector.tensor_copy(
                out=v6_sb[t].rearrange("p (h c) -> p h c", c=128)[:, :, 0:HD + 1],
                in_=ps.rearrange("p (h c) -> p h c", c=HD + 1))

        # all projections ahead of attention (attention is ACT/PE-balanced,
        # so interleaving prework into it just stretches the exp pipeline)
        for t in range(3):
            for nb in range(4):
                qk_chunk(wq_sb, wqb_sb, qTz, t, nb)
                qk_chunk(wk_sb, wkb_sb, kTz, t, nb)
        for t in range(ST):
            v_tile(t)

        sums6 = pp.tile([HPC, S], F32, name="sums6", tag="sums6")

        # ---------------- attention, head by head -----------------------
        for h in range(HPC):
            ctx_ps = ctx_pool.tile([128, S], F32, tag="ctx", name=f"ctx{h}")
            for kt in range(KT):
                mt = mask_pool.tile([128, S], BBF, tag="m", name=f"m{h}_{kt}")
                dma_eng = nc.sync if kt % 2 == 0 else nc.gpsimd
                dma_eng.dma_start(out=mt, in_=mT[h, kt])
                for qh in range(2):
                    st = ps_pool.tile([128, 1024], F32, tag="work",
                                      name=f"st{h}_{kt}_{qh}")
                    for i in range(2):
                        q0 = qh * 1024 + i * 512
                        nc.tensor.matmul(
                            st[:, i * 512:(i + 1) * 512],
                            lhsT=kTz[h][:, kt * 128:(kt + 1) * 128],
                            rhs=qTz[h][:, q0:q0 + 512],
                            start=True, stop=True)
                    p = p_pool.tile([128, 1024], BBF, tag="p",
                                    name=f"p{h}_{kt}_{qh}")
                    nc.scalar.activation(p, st, EXP, scale=1.0 / math.sqrt(HD))
                    nc.vector.tensor_tensor(
                        p, p, mt[:, qh * 1024:(qh + 1) * 1024], MULT)
                    for i in range(2):
                        q0 = qh * 1024 + i * 512
                        nc.tensor.matmul(
                            ctx_ps[:, q0:q0 + 512],
                            lhsT=v6_sb[kt][:, h * 128:(h + 1) * 128],
                            rhs=p[:, i * 512:(i + 1) * 512],
                            start=(kt == 0), stop=(kt == KT - 1))
            # head epilogue: ctxT+sums row to SBUF, then gather the bf16
            # sums row into the f32 sums6 row h via a casting SWDGE DMA
            # (no engine cost, any partition).
            nc.vector.tensor_copy(out=ctxu_sb[h], in_=ctx_ps[0:65, :])
            nc.gpsimd.dma_start(out=sums6[h:h + 1, :],
                                in_=ctxu_sb[h][64:65, :])

        # keep the PE array busy through the normalization latency chain so
        # the HAM clock gate doesn't re-throttle before the out-projection
        warm2_ps = ps_pool.tile([128, 512], F32, tag="work", name="warm2_ps")
        for _ in range(48):
            nc.tensor.matmul(warm2_ps, lhsT=warm_sb[:, 0:128], rhs=warm_sb,
                             start=True, stop=True)

        # ---------------- batched softmax normalization ------------------
        # one 6-lane DVE reciprocal, bf16 convert, then per-head selector
        # matmul broadcast (sel has a single 1 per 64-column block) and a
        # normalize multiply
        recipf = pp.tile([HPC, S], F32, name="recipf", tag="recipf")
        nc.vector.reciprocal_approx_fast(out=recipf, in_=sums6)
        recipb = pp.tile([128, S], BBF, name="recipb", tag="recipb")
        nc.vector.memset(recipb, 0.0)
        nc.vector.tensor_copy(out=recipb[0:HPC, :], in_=recipf)
        for h in range(HPC):
            t, ro = h // 2, (h % 2) * 64
            for qh in range(2):
                rb = ps_pool.tile([64, 1024], F32, tag="work",
                                  name=f"rb{h}_{qh}")
                for i in range(2):
                    q0 = qh * 1024 + i * 512
                    nc.tensor.matmul(
                        rb[:, i * 512:(i + 1) * 512],
                        lhsT=sel_sb[:, h * 64:(h + 1) * 64],
                        rhs=recipb[:, q0:q0 + 512],
                        start=True, stop=True)
                nc.vector.tensor_tensor(
                    ctxa_sb[t][ro:ro + 64, qh * 1024:(qh + 1) * 1024],
                    ctxu_sb[h][0:64, qh * 1024:(qh + 1) * 1024],
                    rb, MULT)

        # ---------------- out projection (partial over 384 features) ----
        for si in range(ST):
            if si % 3 == 2:
                op = ctx_pool.tile([128, D], F32, tag="ctx", name=f"ops{si}")
            else:
                op = ps_pool.tile([128, D], F32, tag="work", name=f"ops{si}")
            for c in range(3):
                nc.tensor.matmul(
                    op[:, 0:512],
                    lhsT=ctxa_sb[c][:, si * 128:(si + 1) * 128],
                    rhs=ow_sb[c][:, 0:512],
                    start=(c == 0), stop=(c == 2))
                nc.tensor.matmul(
                    op[:, 512:D],
                    lhsT=ctxa_sb[c][:, si * 128:(si + 1) * 128],
                    rhs=ow_sb[c][:, 512:D],
                    start=(c == 0), stop=(c == 2))
            ot = o_pool.tile([128, D], BBF, tag="o", name=f"ot{si}")
            if si % 2 == 0:
                nc.vector.tensor_copy(out=ot, in_=op)
            else:
                nc.scalar.copy(out=ot, in_=op)
            nc.sync.dma_start(out=out[si * 128:(si + 1) * 128, :], in_=ot)

    return nc


def _get_nc(finalized=False):
    if "nc" not in _CACHE:
        _CACHE["nc"] = _build_bass()
    nc = _CACHE["nc"]
    if finalized and not nc.is_finalized():
        nc.finalize()
    return nc


def _prep_core_inputs(inputs, core):
    """Host-side shard prep for one core: slice + transpose + bf16."""
    hs = np.asarray(inputs["hidden_states"], np.float32)
    mask = np.asarray(inputs["attention_mask"])
    q_w = np.asarray(inputs["q_w"], np.float32)
    q_b = np.asarray(inputs["q_b"], np.float32)
    k_w = np.asarray(inputs["k_w"], np.float32)
    k_b = np.asarray(inputs["k_b"], np.float32)
    v_w = np.asarray(inputs["v_w"], np.float32)
    v_b = np.asarray(inputs["v_b"], np.float32)
    out_w = np.asarray(inputs["out_w"], np.float32)

    b, hh = divmod(core, 2)
    hsl = slice(hh * FPC, (hh + 1) * FPC)

    hsT_aug = np.empty((D + 1, S), np.float32)
    hsT_aug[:D] = hs[b].T
    hsT_aug[D] = 1.0

    def aug_T(w, bias):
        a = np.empty((D + 1, FPC), np.float32)
        a[:D] = w[hsl].T
        a[D] = bias[hsl]
        return a

    wv = np.zeros((D + 1, HPC * (HD + 1)), np.float32)
    for j in range(HPC):
        fs = hh * FPC + j * HD
        wv[0:D, j * (HD + 1):j * (HD + 1) + HD] = v_w[fs:fs + HD].T
        wv[D, j * (HD + 1):j * (HD + 1) + HD] = v_b[fs:fs + HD]
        wv[D, j * (HD + 1) + HD] = 1.0

    sel = np.zeros((128, FPC), np.float32)
    for j in range(HPC):
        sel[j, j * HD:(j + 1) * HD] = 1.0

    heads = slice(hh * HPC, (hh + 1) * HPC)
    mT6 = np.ascontiguousarray(
        mask[0, heads].transpose(0, 2, 1)).reshape(HPC, KT, 128, S)

    return {
        "hsT": hsT_aug.astype(BF16),
        "wqT": aug_T(q_w, q_b).astype(BF16),
        "wkT": aug_T(k_w, k_b).astype(BF16),
        "wvT": wv.astype(BF16),
        "owT": np.ascontiguousarray(out_w[:, hsl].T).astype(BF16),
        "selp": sel.astype(BF16),
        "mT": mT6.astype(BF16),
    }


def kernel(**inputs):
    global _last_result
    nc = _get_nc(finalized=True)
    in_maps = [_prep_core_inputs(inputs, c) for c in range(NCORES)]
    res = run_bass_kernel_spmd(
        nc, in_maps, core_ids=list(range(NCORES)),
        tmpdir=os.environ.get("KERNEL_TRACE_DIR") or None)
    _last_result = res
    outs = [np.asarray(r["out"], dtype=np.float32) for r in res.results]
    out_b = np.asarray(inputs["out_b"], np.float32)
    full = np.empty((B, S, D), np.float32)
    for b in range(B):
        full[b] = outs[2 * b] + outs[2 * b + 1] + out_b
    return full


# revision 44
# speedup vs baseline: 1.0583x; 1.0583x over previous
"""Trainium2 Bass kernel for masked multi-head attention.

Problem: B=4, S=2048, D=768, H=12 (head_dim=64), boolean prune mask per
head, softmax over keys, out-projection.

Sharding (8 cores): data-parallel over batch (4) x tensor-parallel over
head halves (2 x 6 heads).  Core c handles batch c//2 and heads
(c%2)*6 .. (c%2)*6+5.  Each core computes its 6 heads' QKV projections,
attention, and the partial out-projection (row-parallel slice of out_w).
The host sums the two partials per batch and adds out_b (the standard
tensor-parallel reduce, done during unshard).

On-chip layout choices:
  * Activations are kept feature-major ("transposed"): hsT [769, 2048]
    (row 768 = ones for the affine/bias trick), qT/kT [384, 2048].
  * Scores are computed directly transposed: S_T[k, q] = kT.T-slice @ qT,
    so P.T is exactly the rhs the ctx matmul needs -> no transposes.
  * The mask is pre-transposed to [k, q] per head on the host (bf16 0/1),
    applied multiplicatively after exp (equivalent to -inf before
    softmax, and safe: |scores/8| < ~2 so no overflow without max-sub).
  * V carries an appended ones column per head (wvT has 6x65 columns)
    so row 64 of each ctx PSUM accumulates the softmax denominators.
  * Normalization: denominators gathered per head via a casting SWDGE
    DMA, one batched DVE reciprocal at the end, broadcast across each
    head's 64 partitions with a selector matmul, then DVE multiplies.
  * All matmul inputs bf16 (PSUM accumulates f32); output stored bf16.
  * Scores/ctx stationaries zero-padded to K=128 / M=128 (half-active
    PE arrays made the HAM clock gate hold the PE at 1.2 GHz), plus a
    PE warm-up spin during the initial DMAs.
"""

import os
import sys
import math

import numpy as np

try:
    import concourse.bass as bass
except ImportError:  # pragma: no cover - path fallback for fresh dirs
    for _p in ("/opt/trn_rl_repo", "/root/.axon_site/_ro/trn_rl_repo"):
        if os.path.isdir(_p) and _p not in sys.path:
            sys.path.insert(0, _p)
    import concourse.bass as bass

import ml_dtypes
import concourse.mybir as mybir
from concourse import bacc
from concourse.tile import TileContext
from concourse.bass_utils import run_bass_kernel_spmd

BF16 = ml_dtypes.bfloat16
F32 = mybir.dt.float32
BBF = mybir.dt.bfloat16

B, S, D, H = 4, 2048, 768, 12
HD = 64          # head dim
HPC = 6          # heads per core
FPC = HPC * HD   # features per core (384)
NCORES = 8
KT = S // 128    # 16 key tiles
ST = S // 128    # 16 seq tiles

_CACHE = {}
_last_result = None


def _build_bass():
    nc = bacc.Bacc()

    hsT = nc.declare_dram_parameter("hsT", [D + 1, S], BBF, isOutput=False)
    wqT = nc.declare_dram_parameter("wqT", [D + 1, FPC], BBF, isOutput=False)
    wkT = nc.declare_dram_parameter("wkT", [D + 1, FPC], BBF, isOutput=False)
    wvT = nc.declare_dram_parameter("wvT", [D + 1, HPC * (HD + 1)], BBF, isOutput=False)
    owT = nc.declare_dram_parameter("owT", [FPC, D], BBF, isOutput=False)
    selp = nc.declare_dram_parameter("selp", [128, FPC], BBF, isOutput=False)
    mT = nc.declare_dram_parameter("mT", [HPC, KT, 128, S], BBF, isOutput=False)
    out = nc.declare_dram_parameter("out", [S, D], BBF, isOutput=True)

    EXP = mybir.ActivationFunctionType.Exp
    LN = mybir.ActivationFunctionType.Ln
    MULT = mybir.AluOpType.mult

    with TileContext(nc) as tc, \
            tc.tile_pool(name="persist", bufs=1) as pp, \
            tc.tile_pool(name="maskp", bufs=3) as mask_pool, \
            tc.tile_pool(name="pbuf", bufs=3) as p_pool, \
            tc.tile_pool(name="obuf", bufs=2) as o_pool, \
            tc.tile_pool(name="pswork", bufs=2, space="PSUM") as ps_pool, \
            tc.tile_pool(name="psctx", bufs=1, space="PSUM") as ctx_pool:

        # ---------------- persistent SBUF tensors + input DMAs ----------
        hsT_sb = [pp.tile([128, S], BBF, name=f"hsT{c}", tag=f"hsT{c}")
                  for c in range(6)]
        # all-ones row: content is uniform, so a [1, 512] tile serves every
        # 512-wide rhs slice and every 128-wide lhsT slice
        ones_sb = pp.tile([1, 512], BBF, name="ones_row", tag="ones_row")
        for c in range(6):
            eng = nc.sync if c % 2 == 0 else nc.scalar
            eng.dma_start(out=hsT_sb[c], in_=hsT[c * 128:(c + 1) * 128, :])
        nc.sync.dma_start(out=ones_sb, in_=hsT[D:D + 1, 0:512])

        def load_w(handle, width, nm):
            tiles = [pp.tile([128, width], BBF, name=f"{nm}{c}", tag=f"{nm}{c}")
                     for c in range(6)]
            brow = pp.tile([1, width], BBF, name=f"{nm}b", tag=f"{nm}b")
            for c in range(6):
                eng = nc.scalar if c % 2 == 0 else nc.sync
                eng.dma_start(out=tiles[c],
                              in_=handle[c * 128:(c + 1) * 128, :])
            nc.scalar.dma_start(out=brow, in_=handle[D:D + 1, :])
            return tiles, brow

        wq_sb, wqb_sb = load_w(wqT, FPC, "wq")
        wk_sb, wkb_sb = load_w(wkT, FPC, "wk")
        wv_sb, wvb_sb = load_w(wvT, HPC * (HD + 1), "wv")

        ow_sb = [pp.tile([128, D], BBF, name=f"ow{c}", tag=f"ow{c}")
                 for c in range(3)]
        for c in range(3):
            nc.scalar.dma_start(out=ow_sb[c], in_=owT[c * 128:(c + 1) * 128, :])
        sel_sb = pp.tile([128, FPC], BBF, name="sel", tag="sel")
        nc.scalar.dma_start(out=sel_sb, in_=selp[:, :])

        # Per-head zero-padded qT/kT [128, S]: rows 0-63 = head features,
        # rows 64-127 = 0.  K=128 scores matmuls keep the PE array fully
        # active (K=64 ran at HAM half-clock) and zeros contribute nothing.
        qTz = [pp.tile([128, S], BBF, name=f"qTz{h}", tag=f"qTz{h}")
               for h in range(HPC)]
        kTz = [pp.tile([128, S], BBF, name=f"kTz{h}", tag=f"kTz{h}")
               for h in range(HPC)]
        # v6pad [128, 6*128]: head h occupies cols h*128..h*128+64 (64 v cols
        # + ones col), cols 65-127 of each block zero -> ctx lhsT is a full
        # [128, 128] stationary.
        v6_sb = [pp.tile([128, HPC * 128], BBF, name=f"v6_{t}", tag=f"v6_{t}")
                 for t in range(ST)]
        # PE warm-up: dummy matmuls on (uninitialized, never-read) data so
        # the HAM clock gate reaches 8/8 while the initial DMAs land.  No
        # deps -> starts right after the preamble.
        warm_sb = pp.tile([128, 512], BBF, name="warm_sb", tag="warm_sb")
        nc.vector.memset(warm_sb, 0.0)
        warm_ps = ps_pool.tile([128, 512], F32, tag="work", name="warm_ps")
        for _ in range(56):
            nc.tensor.matmul(warm_ps, lhsT=warm_sb[:, 0:128], rhs=warm_sb,
                             start=True, stop=True)
        nc.vector.tensor_copy(out=warm_sb[:, 0:1], in_=warm_ps[:, 0:1])

        # Zero only what must be zero, off the DVE critical path:
        # qTz/kTz rows 64-127 on GpSimd (head order, so head 0 unblocks
        # first); v6's 63-wide per-head pad columns via a tiny strided DVE
        # memset.
        for h in range(HPC):
            nc.gpsimd.memset(qTz[h][64:128, :], 0.0)
            nc.gpsimd.memset(kTz[h][64:128, :], 0.0)
        for t in range(ST):
            nc.vector.memset(
                v6_sb[t].rearrange("p (h c) -> p h c", c=128)[:, :, HD + 1:], 0.0)
        ctxu_sb = [pp.tile([65, S], BBF, name=f"ctxu{h}", tag=f"ctxu{h}")
                   for h in range(HPC)]
        ctxa_sb = [pp.tile([128, S], BBF, name=f"ctxa{t}", tag=f"ctxa{t}")
                   for t in range(3)]

        # ---------------- projection emitters ----------------------------
        _prew = [0]

        def _prew_tile(nm):
            # rotate projection psums over 3 slots: the 2 "work" slots plus
            # the (idle until attention) ctx slot -> denser prework PE
            _prew[0] += 1
            if _prew[0] % 3 == 0:
                return ctx_pool.tile([128, 512], F32, tag="ctx", name=nm)
            return ps_pool.tile([128, 512], F32, tag="work", name=nm)

        def qk_chunk(w_tiles, w_brow, dst, t, nb):
            # one [128, 512] psum tile of the q or k projection for feature
            # M-tile t (heads 2t, 2t+1), seq block nb; split-copy the two
            # heads' 64-row halves into their zero-padded dsts.
            ps = _prew_tile(f"qkps{id(dst)}_{t}_{nb}")
            for c in range(6):
                nc.tensor.matmul(
                    ps,
                    lhsT=w_tiles[c][:, t * 128:(t + 1) * 128],
                    rhs=hsT_sb[c][:, nb * 512:(nb + 1) * 512],
                    start=(c == 0), stop=False)
            nc.tensor.matmul(
                ps,
                lhsT=w_brow[:, t * 128:(t + 1) * 128],
                rhs=ones_sb,
                start=False, stop=True)
            ns = slice(nb * 512, (nb + 1) * 512)
            nc.vector.tensor_copy(out=dst[2 * t][0:64, ns], in_=ps[0:64, :])
            nc.scalar.copy(out=dst[2 * t + 1][0:64, ns], in_=ps[64:128, :])

        def v_tile(t):
            VW = HPC * (HD + 1)  # 390
            ps = _prew_tile(f"vps{t}")[:, 0:VW]
            for c in range(6):
                nc.tensor.matmul(
                    ps,
                    lhsT=hsT_sb[c][:, t * 128:(t + 1) * 128],
                    rhs=wv_sb[c],
                    start=(c == 0), stop=False)
            nc.tensor.matmul(
                ps,
                lhsT=ones_sb[:, 0:128],
                rhs=wvb_sb,
                start=False, stop=True)
            # scatter [128, 6, 65] -> cols 0..64 of each 128-wide head block
            nc.vector.tensor_copy(
                out=v6_sb[t].rearrange("p (h c) -> p h c", c=128)[:, :, 0:HD + 1],
                in_=ps.rearrange("p (h c) -> p h c", c=HD + 1))

        # all projections ahead of attention (attention is ACT/PE-balanced,
        # so interleaving prework into it just stretches the exp pipeline)
        for t in range(3):
            for nb in range(4):
                qk_chunk(wq_sb, wqb_sb, qTz, t, nb)
                qk_chunk(wk_sb, wkb_sb, kTz, t, nb)
        for t in range(ST):
            v_tile(t)

        sums6 = pp.tile([HPC, S], F32, name="sums6", tag="sums6")

        # ---------------- attention, head by head -----------------------
        for h in range(HPC):
            ctx_ps = ctx_pool.tile([128, S], F32, tag="ctx", name=f"ctx{h}")
            for kt in range(KT):
                mt = mask_pool.tile([128, S], BBF, tag="m", name=f"m{h}_{kt}")
                dma_eng = nc.sync if kt % 2 == 0 else nc.gpsimd
                dma_eng.dma_start(out=mt, in_=mT[h, kt])
                # emit both q-halves' scores before any ctx so the two exps
                # run back-to-back on ACT (ctx in the PE FIFO between them
                # serialized exp1 behind mult0 <- exp0, ~1.2us/kt)
                sts, ps = [], []
                for qh in range(2):
                    st = ps_pool.tile([128, 1024], F32, tag="work",
                                      name=f"st{h}_{kt}_{qh}")
                    for i in range(2):
                        q0 = qh * 1024 + i * 512
                        nc.tensor.matmul(
                            st[:, i * 512:(i + 1) * 512],
                            lhsT=kTz[h][:, kt * 128:(kt + 1) * 128],
                            rhs=qTz[h][:, q0:q0 + 512],
                            start=True, stop=True)
                    sts.append(st)
                for qh in range(2):
                    p = p_pool.tile([128, 1024], BBF, tag="p",
                                    name=f"p{h}_{kt}_{qh}")
                    nc.scalar.activation(p, sts[qh], EXP,
                                         scale=1.0 / math.sqrt(HD))
                    nc.vector.tensor_tensor(
                        p, p, mt[:, qh * 1024:(qh + 1) * 1024], MULT)
                    ps.append(p)
                for qh in range(2):
                    for i in range(2):
                        q0 = qh * 1024 + i * 512
                        nc.tensor.matmul(
                            ctx_ps[:, q0:q0 + 512],
                            lhsT=v6_sb[kt][:, h * 128:(h + 1) * 128],
                            rhs=ps[qh][:, i * 512:(i + 1) * 512],
                            start=(kt == 0), stop=(kt == KT - 1))
            # head epilogue: ctxT+sums row to SBUF, then gather the bf16
            # sums row into the f32 sums6 row h via a casting SWDGE DMA
            # (no engine cost, any partition).
            nc.vector.tensor_copy(out=ctxu_sb[h], in_=ctx_ps[0:65, :])
            nc.gpsimd.dma_start(out=sums6[h:h + 1, :],
                                in_=ctxu_sb[h][64:65, :])

        # keep the PE array busy through the normalization latency chain so
        # the HAM clock gate doesn't re-throttle before the out-projection
        warm2_ps = ps_pool.tile([128, 512], F32, tag="work", name="warm2_ps")
        for _ in range(48):
            nc.tensor.matmul(warm2_ps, lhsT=warm_sb[:, 0:128], rhs=warm_sb,
                             start=True, stop=True)

        # ---------------- batched softmax normalization ------------------
        # one 6-lane DVE reciprocal, bf16 convert, then per-head selector
        # matmul broadcast (sel has a single 1 per 64-column block) and a
        # normalize multiply
        recipf = pp.tile([HPC, S], F32, name="recipf", tag="recipf")
        nc.vector.reciprocal_approx_fast(out=recipf, in_=sums6)
        recipb = pp.tile([128, S], BBF, name="recipb", tag="recipb")
        nc.vector.memset(recipb, 0.0)
        nc.vector.tensor_copy(out=recipb[0:HPC, :], in_=recipf)
        for h in range(HPC):
            t, ro = h // 2, (h % 2) * 64
            for qh in range(2):
                rb = ps_pool.tile([64, 1024], F32, tag="work",
                                  name=f"rb{h}_{qh}")
                for i in range(2):
                    q0 = qh * 1024 + i * 512
                    nc.tensor.matmul(
                        rb[:, i * 512:(i + 1) * 512],
                        lhsT=sel_sb[:, h * 64:(h + 1) * 64],
                        rhs=recipb[:, q0:q0 + 512],
                        start=True, stop=True)
                nc.vector.tensor_tensor(
                    ctxa_sb[t][ro:ro + 64, qh * 1024:(qh + 1) * 1024],
                    ctxu_sb[h][0:64, qh * 1024:(qh + 1) * 1024],
                    rb, MULT)

        # ---------------- out projection (partial over 384 features) ----
        for si in range(ST):
            if si % 3 == 2:
                op = ctx_pool.tile([128, D], F32, tag="ctx", name=f"ops{si}")
            else:
                op = ps_pool.tile([128, D], F32, tag="work", name=f"ops{si}")
            for c in range(3):
                nc.tensor.matmul(
                    op[:, 0:512],
                    lhsT=ctxa_sb[c][:, si * 128:(si + 1) * 128],
                    rhs=ow_sb[c][:, 0:512],
                    start=(c == 0), stop=(c == 2))
                nc.tensor.matmul(
                    op[:, 512:D],
                    lhsT=ctxa_sb[c][:, si * 128:(si + 1) * 128],
                    rhs=ow_sb[c][:, 512:D],
                    start=(c == 0), stop=(c == 2))
            ot = o_pool.tile([128, D], BBF, tag="o", name=f"ot{si}")
            if si % 2 == 0:
                nc.vector.tensor_copy(out=ot, in_=op)
            else:
                nc.scalar.copy(out=ot, in_=op)
            nc.sync.dma_start(out=out[si * 128:(si + 1) * 128, :], in_=ot)

    return nc


def _get_nc(finalized=False):
    if "nc" not in _CACHE:
        _CACHE["nc"] = _build_bass()
    nc = _CACHE["nc"]
    if finalized and not nc.is_finalized():
        nc.finalize()
    return nc


def _prep_core_inputs(inputs, core):
    """Host-side shard prep for one core: slice + transpose + bf16."""
    hs = np.asarray(inputs["hidden_states"], np.float32)
    mask = np.asarray(inputs["attention_mask"])
    q_w = np.asarray(inputs["q_w"], np.float32)
    q_b = np.asarray(inputs["q_b"], np.float32)
    k_w = np.asarray(inputs["k_w"], np.float32)
    k_b = np.asarray(inputs["k_b"], np.float32)
    v_w = np.asarray(inputs["v_w"], np.float32)
    v_b = np.asarray(inputs["v_b"], np.float32)
    out_w = np.asarray(inputs["out_w"], np.float32)

    b, hh = divmod(core, 2)
    hsl = slice(hh * FPC, (hh + 1) * FPC)

    hsT_aug = np.empty((D + 1, S), np.float32)
    hsT_aug[:D] = hs[b].T
    hsT_aug[D] = 1.0

    def aug_T(w, bias):
        a = np.empty((D + 1, FPC), np.float32)
        a[:D] = w[hsl].T
        a[D] = bias[hsl]
        return a

    wv = np.zeros((D + 1, HPC * (HD + 1)), np.float32)
    for j in range(HPC):
        fs = hh * FPC + j * HD
        wv[0:D, j * (HD + 1):j * (HD + 1) + HD] = v_w[fs:fs + HD].T
        wv[D, j * (HD + 1):j * (HD + 1) + HD] = v_b[fs:fs + HD]
        wv[D, j * (HD + 1) + HD] = 1.0

    sel = np.zeros((128, FPC), np.float32)
    for j in range(HPC):
        sel[j, j * HD:(j + 1) * HD] = 1.0

    heads = slice(hh * HPC, (hh + 1) * HPC)
    mT6 = np.ascontiguousarray(
        mask[0, heads].transpose(0, 2, 1)).reshape(HPC, KT, 128, S)

    return {
        "hsT": hsT_aug.astype(BF16),
        "wqT": aug_T(q_w, q_b).astype(BF16),
        "wkT": aug_T(k_w, k_b).astype(BF16),
        "wvT": wv.astype(BF16),
        "owT": np.ascontiguousarray(out_w[:, hsl].T).astype(BF16),
        "selp": sel.astype(BF16),
        "mT": mT6.astype(BF16),
    }


def kernel(**inputs):
    global _last_result
    nc = _get_nc(finalized=True)
    in_maps = [_prep_core_inputs(inputs, c) for c in range(NCORES)]
    res = run_bass_kernel_spmd(
        nc, in_maps, core_ids=list(range(NCORES)),
        tmpdir=os.environ.get("KERNEL_TRACE_DIR") or None)
    _last_result = res
    outs = [np.asarray(r["out"], dtype=np.float32) for r in res.results]
    out_b = np.asarray(inputs["out_b"], np.float32)
    full = np.empty((B, S, D), np.float32)
    for b in range(B):
        full[b] = outs[2 * b] + outs[2 * b + 1] + out_b
    return full
